# revision 14
# baseline (speedup 1.0000x reference)
"""Darknet 3x3 conv block (conv * mask + bias) on 8 TRN2 NeuronCores.

Problem: x[1,512,192,192] (*) w[512,512,3,3] stride1 pad1, then *mask + bias.

Strategy: mixed Winograd, vertical F(2,3) x horizontal F(4,3) -- 2x4 output
tiles, 24 taps per 8 outputs = 3 PE MACs per output-pixel-channel (dense = 9,
F(2x2,3x3) = 4).

  - Host: input transform x~ = B2^T d B4 over 4x6 input tiles (stride 2x4),
    weight transform w~ = G2 w G4^T; both f32, shipped bf16.  Spatial shard
    over H: core k owns 24 output rows = 12 tile-rows x 48 tile-cols = 576
    tiles, split into 3 chunks of 192 tiles.
  - Device per (chunk, fm): 6 psum groups (one per horizontal tap b), each
    a 2-bank PSUM tile accumulating m[a,b] over 4 c-chunks (16 matmuls of
    [c128 x 192]).  ScalarE drains PSUM -> SBUF bf16; DVE does the output
    transform at bf16 2x: stage1 u = A2^T m (4 ops/group), stage2 y = u A4
    (10 ops per row i, using scalar_tensor_tensor for the 2/4/8-weighted
    terms).  GpSimd (otherwise idle) applies mask and bias; y ships bf16.
  - Engine budget per chunk-fm: PE ~8.6us, DVE ~7.1us, ACT ~5.6us,
    GpSimd ~4.6us, DMA ~7.5us -> PE-bound near the F43 roofline
    (~92us/core matmul + overheads).
"""

import sys

for _p in ("/opt/trn_rl_repo",):
    if _p not in sys.path:
        sys.path.insert(0, _p)

import numpy as np
import ml_dtypes

N_CORES = 8
C = 512
F = 512
H = 192
W = 192
HC = H // N_CORES          # output rows per core = 24
TH = HC // 2               # tile-rows per core = 12
TW = W // 4                # tile-cols = 48
CC = C // 128              # c chunks = 4
FM = F // 128              # f chunks = 4
NB = 6                     # horizontal taps
TAPS = 4 * NB              # 24 taps, tap = 4*b + a
CHUNK = 192                # tiles per chunk (4 tile-rows x 48)
NCH = (TH * TW) // CHUNK   # chunks per core = 3
NWARM = 16                 # PE warmup matmuls while first DMAs land

_CACHE = {}


def _build():
    import concourse.bacc as bacc
    import concourse.mybir as mybir
    from concourse.tile import TileContext

    BF = mybir.dt.bfloat16
    F32 = mybir.dt.float32
    MULT = mybir.AluOpType.mult
    ADD = mybir.AluOpType.add

    nc = bacc.Bacc(trn_type="TRN2", num_devices=N_CORES)
    xt_sh = nc.dram_tensor("xt_sh", [128, NCH, CC, TAPS, CHUNK], BF,
                           kind="ExternalInput")
    wt_sh = nc.dram_tensor("wt_sh", [128, FM, CC, TAPS, 128], BF,
                           kind="ExternalInput")
    mk_sh = nc.dram_tensor("mk_sh", [128, NCH, 2, 4, CHUNK], BF,
                           kind="ExternalInput")
    b_sh = nc.dram_tensor("b_sh", [128, FM], F32, kind="ExternalInput")
    y_sh = nc.dram_tensor("y_sh", [NCH, FM, 128, 2, 4, CHUNK], BF,
                          kind="ExternalOutput")

    with TileContext(nc) as tc:
        with (
            tc.tile_pool(name="const", bufs=1) as cpool,
            tc.tile_pool(name="xin", bufs=2) as xpool,
            tc.tile_pool(name="mkp", bufs=2) as mkpool,
            tc.tile_pool(name="psum", bufs=2, space="PSUM") as ppool,
            tc.tile_pool(name="mcp", bufs=3) as mpool,
            tc.tile_pool(name="ust", bufs=2) as upool,
            tc.tile_pool(name="ttp", bufs=2) as tpool,
            tc.tile_pool(name="yst", bufs=3) as ypool,
        ):
            # PE warmup while the first DMAs land (HAM pre-warm + head fill)
            scratch = cpool.tile([128, 256], BF)
            nc.vector.memset(scratch[:], 0.0)
            wps = ppool.tile([128, 4, 512], F32, name="warm", tag="ps")
            for _ in range(NWARM):
                nc.tensor.matmul(wps[:, 0, :CHUNK], scratch[:, :128],
                                 scratch[:, :CHUNK], start=True, stop=True)

            # All DMAs on the SP HWDGE ring (ACT queue stays clear for psum
            # drains).  Every slice is per-partition contiguous.
            wt_t = cpool.tile([128, FM, CC, TAPS, 128], BF)

            xts = {}
            mks = {}

            def load_chunk(ch):
                xt = xpool.tile([128, CC, TAPS, CHUNK], BF, name=f"xt{ch}",
                                tag="xt")
                for cc in range(CC):
                    nc.sync.dma_start(out=xt[:, cc], in_=xt_sh[:, ch, cc])
                mk = mkpool.tile([128, 2, 4, CHUNK], BF, name=f"mk{ch}",
                                 tag="mk")
                nc.sync.dma_start(out=mk[:], in_=mk_sh[:, ch])
                xts[ch] = xt
                mks[ch] = mk

            for cc in range(CC):
                nc.sync.dma_start(out=wt_t[:, 0, cc], in_=wt_sh[:, 0, cc])
            load_chunk(0)
            b_t = cpool.tile([128, FM], F32)
            nc.sync.dma_start(out=b_t[:], in_=b_sh[:])
            for fm in range(1, FM):
                nc.sync.dma_start(out=wt_t[:, fm], in_=wt_sh[:, fm])
            load_chunk(1)

            for ch in range(NCH):
                if ch + 2 < NCH:
                    load_chunk(ch + 2)
                xt = xts.pop(ch)
                mk = mks.pop(ch)
                for fm in range(FM):
                    ut = upool.tile([128, NB, 2, CHUNK], BF,
                                    name=f"u_{ch}_{fm}", tag="u")
                    for b in range(NB):
                        pt = ppool.tile([128, 4, 512], F32,
                                        name=f"ps_{ch}_{fm}_{b}", tag="ps")
                        for cc in range(CC):
                            for a in range(4):
                                tap = 4 * b + a
                                nc.tensor.matmul(
                                    pt[:, a, :CHUNK],
                                    wt_t[:, fm, cc, tap],
                                    xt[:, cc, tap],
                                    start=(cc == 0), stop=(cc == CC - 1),
                                )
                        # ScalarE drains PSUM (f32 -> bf16); DVE transforms
                        mt = mpool.tile([128, 4, CHUNK], BF,
                                        name=f"m_{ch}_{fm}_{b}", tag="m")
                        nc.scalar.activation(
                            mt[:], pt[:, :, :CHUNK],
                            mybir.ActivationFunctionType.Identity,
                        )
                        # stage1 (vertical): u[0] = m0+m1+m2 ; u[1] = m1-m2-m3
                        nc.vector.tensor_add(ut[:, b, 0], mt[:, 0], mt[:, 1])
                        nc.vector.tensor_add(ut[:, b, 0], ut[:, b, 0], mt[:, 2])
                        nc.vector.tensor_sub(ut[:, b, 1], mt[:, 1], mt[:, 2])
                        nc.vector.tensor_sub(ut[:, b, 1], ut[:, b, 1], mt[:, 3])
                    # stage2 (horizontal F(4,3)):
                    #   y0 = u0+u1+u2+u3+u4 ; y1 = (u1-u2) + 2(u3-u4)
                    #   y2 = (u1+u2) + 4(u3+u4) ; y3 = (u1-u2) + 8(u3-u4) + u5
                    yt = ypool.tile([128, 2, 4, CHUNK], BF,
                                    name=f"y_{ch}_{fm}", tag="y")
                    for i in range(2):
                        tt = tpool.tile([128, 4, CHUNK], BF,
                                        name=f"t_{ch}_{fm}_{i}", tag="tt")
                        nc.vector.tensor_sub(tt[:, 0], ut[:, 1, i], ut[:, 2, i])
                        nc.vector.tensor_sub(tt[:, 1], ut[:, 3, i], ut[:, 4, i])
                        nc.vector.tensor_add(tt[:, 2], ut[:, 1, i], ut[:, 2, i])
                        nc.vector.tensor_add(tt[:, 3], ut[:, 3, i], ut[:, 4, i])
                        nc.vector.tensor_add(yt[:, i, 0], ut[:, 0, i], tt[:, 2])
                        nc.vector.tensor_add(yt[:, i, 0], yt[:, i, 0], tt[:, 3])
                        nc.vector.scalar_tensor_tensor(
                            yt[:, i, 1], tt[:, 1], 2.0, tt[:, 0], MULT, ADD)
                        nc.vector.scalar_tensor_tensor(
                            yt[:, i, 2], tt[:, 3], 4.0, tt[:, 2], MULT, ADD)
                        nc.vector.scalar_tensor_tensor(
                            yt[:, i, 3], tt[:, 1], 8.0, tt[:, 0], MULT, ADD)
                        nc.vector.tensor_add(yt[:, i, 3], yt[:, i, 3],
                                             ut[:, 5, i])
                    # mask + bias on GpSimd (otherwise idle)
                    nc.gpsimd.tensor_mul(yt[:], yt[:], mk[:])
                    nc.gpsimd.tensor_scalar_add(yt[:], yt[:],
                                                b_t[:, fm:fm + 1])
                    nc.sync.dma_start(out=y_sh[ch, fm], in_=yt[:])

    nc.compile()
    return nc


def _pack(x, w, b, mask):
    x = np.asarray(x, dtype=np.float32)
    w = np.asarray(w, dtype=np.float32)
    b = np.asarray(b, dtype=np.float32)
    mask = np.asarray(mask)

    B2T = np.array([[1, 0, -1, 0],
                    [0, 1, 1, 0],
                    [0, -1, 1, 0],
                    [0, 1, 0, -1]], np.float32)
    B4T = np.array([[4, 0, -5, 0, 1, 0],
                    [0, -4, -4, 1, 1, 0],
                    [0, 4, -4, -1, 1, 0],
                    [0, -2, -1, 2, 1, 0],
                    [0, 2, -1, -2, 1, 0],
                    [0, 4, 0, -5, 0, 1]], np.float32)
    G2 = np.array([[1, 0, 0],
                   [0.5, 0.5, 0.5],
                   [0.5, -0.5, 0.5],
                   [0, 0, 1]], np.float32)
    G4 = np.array([[1 / 4, 0, 0],
                   [-1 / 6, -1 / 6, -1 / 6],
                   [-1 / 6, 1 / 6, -1 / 6],
                   [1 / 24, 1 / 12, 1 / 6],
                   [1 / 24, -1 / 12, 1 / 6],
                   [0, 0, 1]], np.float32)

    xp = np.zeros((C, H + 2, W + 2), np.float32)
    xp[:, 1:-1, 1:-1] = x[0]
    s = xp.strides
    d = np.lib.stride_tricks.as_strided(
        xp, shape=(C, H // 2, TW, 4, 6),
        strides=(s[0], 2 * s[1], 4 * s[2], s[1], s[2]))
    # x~[c, tr, tc, i(vert), j(horz)] f32 -> bf16
    xt = np.einsum("ia,ctuab,jb->ctuij", B2T, d, B4T, optimize=True)
    xt = xt.astype(ml_dtypes.bfloat16)

    # w~[f, c, i, j] -> [c_local(128), fm, cc, tap=4j+i, f_local(128)]
    wt = np.einsum("ia,fcab,jb->fcij", G2, w, G4, optimize=True)
    wt = (wt.reshape(FM, 128, CC, 128, 4, NB)
            .transpose(3, 0, 2, 5, 4, 1)          # [128c, fm, cc, j, i, 128f]
            .reshape(128, FM, CC, TAPS, 128))
    wt = np.ascontiguousarray(wt).astype(ml_dtypes.bfloat16)

    b_re = np.ascontiguousarray(b.reshape(FM, 128).T)  # [128, FM]

    mf = mask.astype(np.float32)

    in_maps = []
    for k in range(N_CORES):
        # x~ core k: tile-rows [12k, 12k+12) -> [128, NCH, CC, TAPS, CHUNK]
        xk = xt[:, TH * k:TH * k + TH]            # [512, 12, 48, 4, 6]
        xk = (xk.reshape(CC, 128, NCH, 4, TW, 4, NB)
                .transpose(1, 2, 0, 6, 5, 3, 4)   # [128, NCH, CC, j, i, 4, 48]
                .reshape(128, NCH, CC, TAPS, CHUNK))
        xk = np.ascontiguousarray(xk)

        # mask rows [24k, 24k+24): pixel (2*(4ch+tr)+i, 4tc+j)
        mkk = (mf[HC * k:HC * k + HC]              # [24, 192]
               .reshape(NCH, 4, 2, TW, 4)
               .transpose(0, 2, 4, 1, 3)           # [NCH, i, j, 4, 48]
               .reshape(1, NCH, 2, 4, CHUNK))
        mkk = np.ascontiguousarray(
            np.broadcast_to(mkk, (128, NCH, 2, 4, CHUNK))
        ).astype(ml_dtypes.bfloat16)

        in_maps.append({"xt_sh": xk, "wt_sh": wt, "mk_sh": mkk,
                        "b_sh": b_re})
    return in_maps


def _unpack(results):
    slabs = []
    for k in range(N_CORES):
        ys = np.asarray(results[k]["y_sh"])       # [NCH, FM, 128, 2, 4, CHUNK]
        ys = (ys.reshape(NCH, FM, 128, 2, 4, 4, TW)
                .transpose(1, 2, 0, 5, 3, 6, 4)   # [FM, 128, NCH, 4, i, 48, j]
                .reshape(F, HC, W))
        slabs.append(ys.astype(np.float32))
    out = np.concatenate(slabs, axis=1)           # [512, 192, 192]
    return out[None]


def _run(inputs, **run_kwargs):
    from concourse.bass_utils import run_bass_kernel_spmd

    if "nc" not in _CACHE:
        _CACHE["nc"] = _build()
    nc = _CACHE["nc"]
    in_maps = _pack(inputs["x"], inputs["w"], inputs["b"], inputs["mask"])
    res = run_bass_kernel_spmd(nc, in_maps, core_ids=list(range(N_CORES)),
                               **run_kwargs)
    return _unpack(res.results), res


def kernel(**inputs):
    out, _ = _run(inputs)
    return out


# revision 15
# speedup vs baseline: 2.5011x; 2.5011x over previous
"""Darknet 3x3 conv block (conv * mask + bias) on 8 TRN2 NeuronCores.

Problem: x[1,512,192,192] (*) w[512,512,3,3] stride1 pad1, then *mask + bias.

Strategy: mixed Winograd, vertical F(2,3) x horizontal F(4,3) -- 2x4 output
tiles, 24 taps per 8 outputs = 3 PE MACs per output-pixel-channel (dense = 9,
F(2x2,3x3) = 4).

  - Host: input transform x~ = B2^T d B4 over 4x6 input tiles (stride 2x4),
    weight transform w~ = G2 w G4^T; both f32, shipped bf16.  Spatial shard
    over H: core k owns 24 output rows = 12 tile-rows x 48 tile-cols = 576
    tiles, split into 3 chunks of 192 tiles.
  - Device per (chunk, fm): 6 psum groups (one per horizontal tap b), each
    a 2-bank PSUM tile accumulating m[a,b] over 4 c-chunks (16 matmuls of
    [c128 x 192]).  ScalarE drains PSUM -> SBUF bf16; DVE does the output
    transform at bf16 2x: stage1 u = A2^T m (4 ops/group), stage2 y = u A4
    (10 ops per row i, using scalar_tensor_tensor for the 2/4/8-weighted
    terms).  GpSimd (otherwise idle) applies mask and bias; y ships bf16.
  - Engine budget per chunk-fm: PE ~8.6us, DVE ~7.1us, ACT ~5.6us,
    GpSimd ~4.6us, DMA ~7.5us -> PE-bound near the F43 roofline
    (~92us/core matmul + overheads).
"""

import sys

for _p in ("/opt/trn_rl_repo",):
    if _p not in sys.path:
        sys.path.insert(0, _p)

import numpy as np
import ml_dtypes

N_CORES = 8
C = 512
F = 512
H = 192
W = 192
HC = H // N_CORES          # output rows per core = 24
TH = HC // 2               # tile-rows per core = 12
TW = W // 4                # tile-cols = 48
CC = C // 128              # c chunks = 4
FM = F // 128              # f chunks = 4
NB = 6                     # horizontal taps
TAPS = 4 * NB              # 24 taps, tap = 4*b + a
CHUNK = 192                # tiles per chunk (4 tile-rows x 48)
NCH = (TH * TW) // CHUNK   # chunks per core = 3
NWARM = 16                 # PE warmup matmuls while first DMAs land

_CACHE = {}


def _build():
    import concourse.bacc as bacc
    import concourse.mybir as mybir
    from concourse.tile import TileContext

    BF = mybir.dt.bfloat16
    F32 = mybir.dt.float32
    MULT = mybir.AluOpType.mult
    ADD = mybir.AluOpType.add

    nc = bacc.Bacc(trn_type="TRN2", num_devices=N_CORES)
    xt_sh = nc.dram_tensor("xt_sh", [128, NCH, CC, TAPS, CHUNK], BF,
                           kind="ExternalInput")
    wt_sh = nc.dram_tensor("wt_sh", [128, FM, CC, TAPS, 128], BF,
                           kind="ExternalInput")
    mk_sh = nc.dram_tensor("mk_sh", [128, NCH, 2, 4, CHUNK], BF,
                           kind="ExternalInput")
    b_sh = nc.dram_tensor("b_sh", [128, FM], F32, kind="ExternalInput")
    y_sh = nc.dram_tensor("y_sh", [NCH, FM, 128, 2, 4, CHUNK], BF,
                          kind="ExternalOutput")

    with TileContext(nc) as tc:
        with (
            tc.tile_pool(name="const", bufs=1) as cpool,
            tc.tile_pool(name="xin", bufs=2) as xpool,
            tc.tile_pool(name="mkp", bufs=2) as mkpool,
            tc.tile_pool(name="psum", bufs=2, space="PSUM") as ppool,
            tc.tile_pool(name="mcp", bufs=3) as mpool,
            tc.tile_pool(name="ust", bufs=2) as upool,
            tc.tile_pool(name="ttp", bufs=2) as tpool,
            tc.tile_pool(name="yst", bufs=3) as ypool,
        ):
            # PE warmup while the first DMAs land (HAM pre-warm + head fill)
            scratch = cpool.tile([128, 256], BF)
            nc.vector.memset(scratch[:], 0.0)
            wps = ppool.tile([128, 4, 512], F32, name="warm", tag="ps")
            for _ in range(NWARM):
                nc.tensor.matmul(wps[:, 0, :CHUNK], scratch[:, :128],
                                 scratch[:, :CHUNK], start=True, stop=True)

            # All DMAs on the SP HWDGE ring (ACT queue stays clear for psum
            # drains).  Every slice is per-partition contiguous.
            wt_t = cpool.tile([128, FM, CC, TAPS, 128], BF)

            xts = {}
            mks = {}

            def load_chunk(ch):
                xt = xpool.tile([128, CC, TAPS, CHUNK], BF, name=f"xt{ch}",
                                tag="xt")
                for cc in range(CC):
                    nc.sync.dma_start(out=xt[:, cc], in_=xt_sh[:, ch, cc])
                mk = mkpool.tile([128, 2, 4, CHUNK], BF, name=f"mk{ch}",
                                 tag="mk")
                nc.sync.dma_start(out=mk[:], in_=mk_sh[:, ch])
                xts[ch] = xt
                mks[ch] = mk

            for cc in range(CC):
                nc.sync.dma_start(out=wt_t[:, 0, cc], in_=wt_sh[:, 0, cc])
            load_chunk(0)
            b_t = cpool.tile([128, FM], F32)
            nc.sync.dma_start(out=b_t[:], in_=b_sh[:])
            for fm in range(1, FM):
                nc.sync.dma_start(out=wt_t[:, fm], in_=wt_sh[:, fm])
            load_chunk(1)

            for ch in range(NCH):
                if ch + 2 < NCH:
                    load_chunk(ch + 2)
                xt = xts.pop(ch)
                mk = mks.pop(ch)
                for fm in range(FM):
                    ut = upool.tile([128, NB, 2, CHUNK], BF,
                                    name=f"u_{ch}_{fm}", tag="u")
                    for b in range(NB):
                        pt = ppool.tile([128, 4, 512], F32,
                                        name=f"ps_{ch}_{fm}_{b}", tag="ps")
                        for cc in range(CC):
                            for a in range(4):
                                tap = 4 * b + a
                                nc.tensor.matmul(
                                    pt[:, a, :CHUNK],
                                    wt_t[:, fm, cc, tap],
                                    xt[:, cc, tap],
                                    start=(cc == 0), stop=(cc == CC - 1),
                                )
                        # ScalarE drains PSUM (f32 -> bf16); DVE transforms
                        mt = mpool.tile([128, 4, CHUNK], BF,
                                        name=f"m_{ch}_{fm}_{b}", tag="m")
                        nc.scalar.activation(
                            mt[:], pt[:, :, :CHUNK],
                            mybir.ActivationFunctionType.Identity,
                        )
                        # stage1 (vertical): u[0] = m0+m1+m2 ; u[1] = m1-m2-m3
                        nc.vector.tensor_add(ut[:, b, 0], mt[:, 0], mt[:, 1])
                        nc.vector.tensor_add(ut[:, b, 0], ut[:, b, 0], mt[:, 2])
                        nc.vector.tensor_sub(ut[:, b, 1], mt[:, 1], mt[:, 2])
                        nc.vector.tensor_sub(ut[:, b, 1], ut[:, b, 1], mt[:, 3])
                    # stage2 (horizontal F(4,3)):
                    #   y0 = u0+u1+u2+u3+u4 ; y1 = (u1-u2) + 2(u3-u4)
                    #   y2 = (u1+u2) + 4(u3+u4) ; y3 = (u1-u2) + 8(u3-u4) + u5
                    yt = ypool.tile([128, 2, 4, CHUNK], BF,
                                    name=f"y_{ch}_{fm}", tag="y")
                    for i in range(2):
                        tt = tpool.tile([128, 4, CHUNK], BF,
                                        name=f"t_{ch}_{fm}_{i}", tag="tt")
                        nc.vector.tensor_sub(tt[:, 0], ut[:, 1, i], ut[:, 2, i])
                        nc.vector.tensor_sub(tt[:, 1], ut[:, 3, i], ut[:, 4, i])
                        nc.vector.tensor_add(tt[:, 2], ut[:, 1, i], ut[:, 2, i])
                        nc.vector.tensor_add(tt[:, 3], ut[:, 3, i], ut[:, 4, i])
                        nc.vector.tensor_add(yt[:, i, 0], ut[:, 0, i], tt[:, 2])
                        nc.vector.tensor_add(yt[:, i, 0], yt[:, i, 0], tt[:, 3])
                        nc.vector.scalar_tensor_tensor(
                            yt[:, i, 1], tt[:, 1], 2.0, tt[:, 0], MULT, ADD)
                        nc.vector.scalar_tensor_tensor(
                            yt[:, i, 2], tt[:, 3], 4.0, tt[:, 2], MULT, ADD)
                        nc.vector.scalar_tensor_tensor(
                            yt[:, i, 3], tt[:, 1], 8.0, tt[:, 0], MULT, ADD)
                        nc.vector.tensor_add(yt[:, i, 3], yt[:, i, 3],
                                             ut[:, 5, i])
                    # mask on DVE, bias on ScalarE (gpsimd is pathologically
                    # slow for these and contends with DVE's SBUF port)
                    nc.vector.tensor_mul(yt[:], yt[:], mk[:])
                    nc.scalar.activation(
                        yt[:], yt[:],
                        mybir.ActivationFunctionType.Identity,
                        bias=b_t[:, fm:fm + 1],
                    )
                    nc.sync.dma_start(out=y_sh[ch, fm], in_=yt[:])

    nc.compile()
    return nc


def _pack(x, w, b, mask):
    x = np.asarray(x, dtype=np.float32)
    w = np.asarray(w, dtype=np.float32)
    b = np.asarray(b, dtype=np.float32)
    mask = np.asarray(mask)

    B2T = np.array([[1, 0, -1, 0],
                    [0, 1, 1, 0],
                    [0, -1, 1, 0],
                    [0, 1, 0, -1]], np.float32)
    B4T = np.array([[4, 0, -5, 0, 1, 0],
                    [0, -4, -4, 1, 1, 0],
                    [0, 4, -4, -1, 1, 0],
                    [0, -2, -1, 2, 1, 0],
                    [0, 2, -1, -2, 1, 0],
                    [0, 4, 0, -5, 0, 1]], np.float32)
    G2 = np.array([[1, 0, 0],
                   [0.5, 0.5, 0.5],
                   [0.5, -0.5, 0.5],
                   [0, 0, 1]], np.float32)
    G4 = np.array([[1 / 4, 0, 0],
                   [-1 / 6, -1 / 6, -1 / 6],
                   [-1 / 6, 1 / 6, -1 / 6],
                   [1 / 24, 1 / 12, 1 / 6],
                   [1 / 24, -1 / 12, 1 / 6],
                   [0, 0, 1]], np.float32)

    xp = np.zeros((C, H + 2, W + 2), np.float32)
    xp[:, 1:-1, 1:-1] = x[0]
    s = xp.strides
    d = np.lib.stride_tricks.as_strided(
        xp, shape=(C, H // 2, TW, 4, 6),
        strides=(s[0], 2 * s[1], 4 * s[2], s[1], s[2]))
    # x~[c, tr, tc, i(vert), j(horz)] f32 -> bf16
    xt = np.einsum("ia,ctuab,jb->ctuij", B2T, d, B4T, optimize=True)
    xt = xt.astype(ml_dtypes.bfloat16)

    # w~[f, c, i, j] -> [c_local(128), fm, cc, tap=4j+i, f_local(128)]
    wt = np.einsum("ia,fcab,jb->fcij", G2, w, G4, optimize=True)
    wt = (wt.reshape(FM, 128, CC, 128, 4, NB)
            .transpose(3, 0, 2, 5, 4, 1)          # [128c, fm, cc, j, i, 128f]
            .reshape(128, FM, CC, TAPS, 128))
    wt = np.ascontiguousarray(wt).astype(ml_dtypes.bfloat16)

    b_re = np.ascontiguousarray(b.reshape(FM, 128).T)  # [128, FM]

    mf = mask.astype(np.float32)

    in_maps = []
    for k in range(N_CORES):
        # x~ core k: tile-rows [12k, 12k+12) -> [128, NCH, CC, TAPS, CHUNK]
        xk = xt[:, TH * k:TH * k + TH]            # [512, 12, 48, 4, 6]
        xk = (xk.reshape(CC, 128, NCH, 4, TW, 4, NB)
                .transpose(1, 2, 0, 6, 5, 3, 4)   # [128, NCH, CC, j, i, 4, 48]
                .reshape(128, NCH, CC, TAPS, CHUNK))
        xk = np.ascontiguousarray(xk)

        # mask rows [24k, 24k+24): pixel (2*(4ch+tr)+i, 4tc+j)
        mkk = (mf[HC * k:HC * k + HC]              # [24, 192]
               .reshape(NCH, 4, 2, TW, 4)
               .transpose(0, 2, 4, 1, 3)           # [NCH, i, j, 4, 48]
               .reshape(1, NCH, 2, 4, CHUNK))
        mkk = np.ascontiguousarray(
            np.broadcast_to(mkk, (128, NCH, 2, 4, CHUNK))
        ).astype(ml_dtypes.bfloat16)

        in_maps.append({"xt_sh": xk, "wt_sh": wt, "mk_sh": mkk,
                        "b_sh": b_re})
    return in_maps


def _unpack(results):
    slabs = []
    for k in range(N_CORES):
        ys = np.asarray(results[k]["y_sh"])       # [NCH, FM, 128, 2, 4, CHUNK]
        ys = (ys.reshape(NCH, FM, 128, 2, 4, 4, TW)
                .transpose(1, 2, 0, 5, 3, 6, 4)   # [FM, 128, NCH, 4, i, 48, j]
                .reshape(F, HC, W))
        slabs.append(ys.astype(np.float32))
    out = np.concatenate(slabs, axis=1)           # [512, 192, 192]
    return out[None]


def _run(inputs, **run_kwargs):
    from concourse.bass_utils import run_bass_kernel_spmd

    if "nc" not in _CACHE:
        _CACHE["nc"] = _build()
    nc = _CACHE["nc"]
    in_maps = _pack(inputs["x"], inputs["w"], inputs["b"], inputs["mask"])
    res = run_bass_kernel_spmd(nc, in_maps, core_ids=list(range(N_CORES)),
                               **run_kwargs)
    return _unpack(res.results), res


def kernel(**inputs):
    out, _ = _run(inputs)
    return out


# revision 16
# speedup vs baseline: 2.7112x; 1.0840x over previous
"""Darknet 3x3 conv block (conv * mask + bias) on 8 TRN2 NeuronCores.

Problem: x[1,512,192,192] (*) w[512,512,3,3] stride1 pad1, then *mask + bias.

Strategy: mixed Winograd, vertical F(2,3) x horizontal F(4,3) -- 2x4 output
tiles, 24 taps per 8 outputs = 3 PE MACs per output-pixel-channel (dense = 9,
F(2x2,3x3) = 4).

  - Host: input transform x~ = B2^T d B4 over 4x6 input tiles (stride 2x4),
    weight transform w~ = G2 w G4^T; both f32, shipped bf16.  Spatial shard
    over H: core k owns 24 output rows = 12 tile-rows x 48 tile-cols = 576
    tiles, split into 3 chunks of 192 tiles.
  - Device per (chunk, fm): 6 psum groups (one per horizontal tap b), each
    a 2-bank PSUM tile accumulating m[a,b] over 4 c-chunks (16 matmuls of
    [c128 x 192]).  ScalarE drains PSUM -> SBUF bf16; DVE does the output
    transform at bf16 2x: stage1 u = A2^T m (4 ops/group), stage2 y = u A4
    (10 ops per row i, using scalar_tensor_tensor for the 2/4/8-weighted
    terms).  GpSimd (otherwise idle) applies mask and bias; y ships bf16.
  - Engine budget per chunk-fm: PE ~8.6us, DVE ~7.1us, ACT ~5.6us,
    GpSimd ~4.6us, DMA ~7.5us -> PE-bound near the F43 roofline
    (~92us/core matmul + overheads).
"""

import sys

for _p in ("/opt/trn_rl_repo",):
    if _p not in sys.path:
        sys.path.insert(0, _p)

import numpy as np
import ml_dtypes

N_CORES = 8
C = 512
F = 512
H = 192
W = 192
HC = H // N_CORES          # output rows per core = 24
TH = HC // 2               # tile-rows per core = 12
TW = W // 4                # tile-cols = 48
CC = C // 128              # c chunks = 4
FM = F // 128              # f chunks = 4
NB = 6                     # horizontal taps
TAPS = 4 * NB              # 24 taps, tap = 4*b + a
CHUNK = 192                # tiles per chunk (4 tile-rows x 48)
NCH = (TH * TW) // CHUNK   # chunks per core = 3
NWARM = 16                 # PE warmup matmuls while first DMAs land

_CACHE = {}


def _build():
    import concourse.bacc as bacc
    import concourse.mybir as mybir
    from concourse.tile import TileContext

    BF = mybir.dt.bfloat16
    F32 = mybir.dt.float32
    MULT = mybir.AluOpType.mult
    ADD = mybir.AluOpType.add

    nc = bacc.Bacc(trn_type="TRN2", num_devices=N_CORES)
    xt_sh = nc.dram_tensor("xt_sh", [128, NCH, TAPS, CC, CHUNK], BF,
                           kind="ExternalInput")
    wt_sh = nc.dram_tensor("wt_sh", [128, FM, CC, TAPS, 128], BF,
                           kind="ExternalInput")
    mk_sh = nc.dram_tensor("mk_sh", [128, NCH, 2, 4, CHUNK], BF,
                           kind="ExternalInput")
    b_sh = nc.dram_tensor("b_sh", [128, FM], F32, kind="ExternalInput")
    y_sh = nc.dram_tensor("y_sh", [NCH, FM, 128, 2, 4, CHUNK], BF,
                          kind="ExternalOutput")

    with TileContext(nc) as tc:
        with (
            tc.tile_pool(name="const", bufs=1) as cpool,
            tc.tile_pool(name="xin", bufs=2) as xpool,
            tc.tile_pool(name="mkp", bufs=2) as mkpool,
            tc.tile_pool(name="psum", bufs=2, space="PSUM") as ppool,
            tc.tile_pool(name="mcp", bufs=3) as mpool,
            tc.tile_pool(name="ust", bufs=2) as upool,
            tc.tile_pool(name="ttp", bufs=2) as tpool,
            tc.tile_pool(name="yst", bufs=3) as ypool,
        ):
            # PE warmup while the first DMAs land (HAM pre-warm + head fill)
            scratch = cpool.tile([128, 256], BF)
            nc.vector.memset(scratch[:], 0.0)
            wps = ppool.tile([128, 4, 512], F32, name="warm", tag="ps")
            for _ in range(NWARM):
                nc.tensor.matmul(wps[:, 0, :CHUNK], scratch[:, :128],
                                 scratch[:, :CHUNK], start=True, stop=True)

            # All DMAs on the SP HWDGE ring (ACT queue stays clear for psum
            # drains).  Every slice is per-partition contiguous.
            wt_t = cpool.tile([128, FM, CC, TAPS, 128], BF)

            xts = {}
            mks = {}

            def load_chunk(ch):
                # per-b-group DMAs in first-use order: group b's whole
                # working set (taps 4b:4b+4, all cc) is one contiguous DMA
                xt = xpool.tile([128, TAPS, CC, CHUNK], BF, name=f"xt{ch}",
                                tag="xt")
                for b in range(NB):
                    nc.sync.dma_start(out=xt[:, 4 * b:4 * b + 4],
                                      in_=xt_sh[:, ch, 4 * b:4 * b + 4])
                mk = mkpool.tile([128, 2, 4, CHUNK], BF, name=f"mk{ch}",
                                 tag="mk")
                nc.sync.dma_start(out=mk[:], in_=mk_sh[:, ch])
                xts[ch] = xt
                mks[ch] = mk

            # JIT order: wt fm0, then x~ ch0 (b-ordered), then later weights
            for cc in range(CC):
                nc.sync.dma_start(out=wt_t[:, 0, cc], in_=wt_sh[:, 0, cc])
            load_chunk(0)
            b_t = cpool.tile([128, FM], F32)
            nc.sync.dma_start(out=b_t[:], in_=b_sh[:])
            nc.sync.dma_start(out=wt_t[:, 1], in_=wt_sh[:, 1])
            load_chunk(1)
            for fm in range(2, FM):
                nc.sync.dma_start(out=wt_t[:, fm], in_=wt_sh[:, fm])

            for ch in range(NCH):
                if 1 <= ch < NCH - 1:
                    # prefetch AFTER the previous chunk's y-outs are queued
                    load_chunk(ch + 1)
                xt = xts.pop(ch)
                mk = mks.pop(ch)
                for fm in range(FM):
                    ut = upool.tile([128, NB, 2, CHUNK], BF,
                                    name=f"u_{ch}_{fm}", tag="u")
                    for b in range(NB):
                        pt = ppool.tile([128, 4, 512], F32,
                                        name=f"ps_{ch}_{fm}_{b}", tag="ps")
                        for cc in range(CC):
                            for a in range(4):
                                tap = 4 * b + a
                                nc.tensor.matmul(
                                    pt[:, a, :CHUNK],
                                    wt_t[:, fm, cc, tap],
                                    xt[:, tap, cc],
                                    start=(cc == 0), stop=(cc == CC - 1),
                                )
                        # ScalarE drains PSUM (f32 -> bf16); DVE transforms
                        mt = mpool.tile([128, 4, CHUNK], BF,
                                        name=f"m_{ch}_{fm}_{b}", tag="m")
                        nc.scalar.activation(
                            mt[:], pt[:, :, :CHUNK],
                            mybir.ActivationFunctionType.Identity,
                        )
                        # stage1 (vertical): u[0] = m0+m1+m2 ; u[1] = m1-m2-m3
                        nc.vector.tensor_add(ut[:, b, 0], mt[:, 0], mt[:, 1])
                        nc.vector.tensor_add(ut[:, b, 0], ut[:, b, 0], mt[:, 2])
                        nc.vector.tensor_sub(ut[:, b, 1], mt[:, 1], mt[:, 2])
                        nc.vector.tensor_sub(ut[:, b, 1], ut[:, b, 1], mt[:, 3])
                    # stage2 (horizontal F(4,3)):
                    #   y0 = u0+u1+u2+u3+u4 ; y1 = (u1-u2) + 2(u3-u4)
                    #   y2 = (u1+u2) + 4(u3+u4) ; y3 = (u1-u2) + 8(u3-u4) + u5
                    yt = ypool.tile([128, 2, 4, CHUNK], BF,
                                    name=f"y_{ch}_{fm}", tag="y")
                    for i in range(2):
                        tt = tpool.tile([128, 4, CHUNK], BF,
                                        name=f"t_{ch}_{fm}_{i}", tag="tt")
                        nc.vector.tensor_sub(tt[:, 0], ut[:, 1, i], ut[:, 2, i])
                        nc.vector.tensor_sub(tt[:, 1], ut[:, 3, i], ut[:, 4, i])
                        nc.vector.tensor_add(tt[:, 2], ut[:, 1, i], ut[:, 2, i])
                        nc.vector.tensor_add(tt[:, 3], ut[:, 3, i], ut[:, 4, i])
                        nc.vector.tensor_add(yt[:, i, 0], ut[:, 0, i], tt[:, 2])
                        nc.vector.tensor_add(yt[:, i, 0], yt[:, i, 0], tt[:, 3])
                        nc.vector.scalar_tensor_tensor(
                            yt[:, i, 1], tt[:, 1], 2.0, tt[:, 0], MULT, ADD)
                        nc.vector.scalar_tensor_tensor(
                            yt[:, i, 2], tt[:, 3], 4.0, tt[:, 2], MULT, ADD)
                        nc.vector.scalar_tensor_tensor(
                            yt[:, i, 3], tt[:, 1], 8.0, tt[:, 0], MULT, ADD)
                        nc.vector.tensor_add(yt[:, i, 3], yt[:, i, 3],
                                             ut[:, 5, i])
                    # mask on DVE, bias on ScalarE (gpsimd is pathologically
                    # slow for these and contends with DVE's SBUF port)
                    nc.vector.tensor_mul(yt[:], yt[:], mk[:])
                    nc.scalar.activation(
                        yt[:], yt[:],
                        mybir.ActivationFunctionType.Identity,
                        bias=b_t[:, fm:fm + 1],
                    )
                    nc.sync.dma_start(out=y_sh[ch, fm], in_=yt[:])

    nc.compile()
    return nc


def _pack(x, w, b, mask):
    x = np.asarray(x, dtype=np.float32)
    w = np.asarray(w, dtype=np.float32)
    b = np.asarray(b, dtype=np.float32)
    mask = np.asarray(mask)

    B2T = np.array([[1, 0, -1, 0],
                    [0, 1, 1, 0],
                    [0, -1, 1, 0],
                    [0, 1, 0, -1]], np.float32)
    B4T = np.array([[4, 0, -5, 0, 1, 0],
                    [0, -4, -4, 1, 1, 0],
                    [0, 4, -4, -1, 1, 0],
                    [0, -2, -1, 2, 1, 0],
                    [0, 2, -1, -2, 1, 0],
                    [0, 4, 0, -5, 0, 1]], np.float32)
    G2 = np.array([[1, 0, 0],
                   [0.5, 0.5, 0.5],
                   [0.5, -0.5, 0.5],
                   [0, 0, 1]], np.float32)
    G4 = np.array([[1 / 4, 0, 0],
                   [-1 / 6, -1 / 6, -1 / 6],
                   [-1 / 6, 1 / 6, -1 / 6],
                   [1 / 24, 1 / 12, 1 / 6],
                   [1 / 24, -1 / 12, 1 / 6],
                   [0, 0, 1]], np.float32)

    xp = np.zeros((C, H + 2, W + 2), np.float32)
    xp[:, 1:-1, 1:-1] = x[0]
    s = xp.strides
    d = np.lib.stride_tricks.as_strided(
        xp, shape=(C, H // 2, TW, 4, 6),
        strides=(s[0], 2 * s[1], 4 * s[2], s[1], s[2]))
    # x~[c, tr, tc, i(vert), j(horz)] f32 -> bf16
    xt = np.einsum("ia,ctuab,jb->ctuij", B2T, d, B4T, optimize=True)
    xt = xt.astype(ml_dtypes.bfloat16)

    # w~[f, c, i, j] -> [c_local(128), fm, cc, tap=4j+i, f_local(128)]
    wt = np.einsum("ia,fcab,jb->fcij", G2, w, G4, optimize=True)
    wt = (wt.reshape(FM, 128, CC, 128, 4, NB)
            .transpose(3, 0, 2, 5, 4, 1)          # [128c, fm, cc, j, i, 128f]
            .reshape(128, FM, CC, TAPS, 128))
    wt = np.ascontiguousarray(wt).astype(ml_dtypes.bfloat16)

    b_re = np.ascontiguousarray(b.reshape(FM, 128).T)  # [128, FM]

    mf = mask.astype(np.float32)

    in_maps = []
    for k in range(N_CORES):
        # x~ core k: tile-rows [12k, 12k+12) -> [128, NCH, CC, TAPS, CHUNK]
        xk = xt[:, TH * k:TH * k + TH]            # [512, 12, 48, 4, 6]
        xk = (xk.reshape(CC, 128, NCH, 4, TW, 4, NB)
                .transpose(1, 2, 6, 5, 0, 3, 4)   # [128, NCH, j, i, CC, 4, 48]
                .reshape(128, NCH, TAPS, CC, CHUNK))
        xk = np.ascontiguousarray(xk)

        # mask rows [24k, 24k+24): pixel (2*(4ch+tr)+i, 4tc+j)
        mkk = (mf[HC * k:HC * k + HC]              # [24, 192]
               .reshape(NCH, 4, 2, TW, 4)
               .transpose(0, 2, 4, 1, 3)           # [NCH, i, j, 4, 48]
               .reshape(1, NCH, 2, 4, CHUNK))
        mkk = np.ascontiguousarray(
            np.broadcast_to(mkk, (128, NCH, 2, 4, CHUNK))
        ).astype(ml_dtypes.bfloat16)

        in_maps.append({"xt_sh": xk, "wt_sh": wt, "mk_sh": mkk,
                        "b_sh": b_re})
    return in_maps


def _unpack(results):
    slabs = []
    for k in range(N_CORES):
        ys = np.asarray(results[k]["y_sh"])       # [NCH, FM, 128, 2, 4, CHUNK]
        ys = (ys.reshape(NCH, FM, 128, 2, 4, 4, TW)
                .transpose(1, 2, 0, 5, 3, 6, 4)   # [FM, 128, NCH, 4, i, 48, j]
                .reshape(F, HC, W))
        slabs.append(ys.astype(np.float32))
    out = np.concatenate(slabs, axis=1)           # [512, 192, 192]
    return out[None]


def _run(inputs, **run_kwargs):
    from concourse.bass_utils import run_bass_kernel_spmd

    if "nc" not in _CACHE:
        _CACHE["nc"] = _build()
    nc = _CACHE["nc"]
    in_maps = _pack(inputs["x"], inputs["w"], inputs["b"], inputs["mask"])
    res = run_bass_kernel_spmd(nc, in_maps, core_ids=list(range(N_CORES)),
                               **run_kwargs)
    return _unpack(res.results), res


def kernel(**inputs):
    out, _ = _run(inputs)
    return out
